# revision 44
# baseline (speedup 1.0000x reference)
"""BEiT-style windowed attention block on 8 TRN2 NeuronCores.

Data-parallel over batch: each core handles 8 of the 64 batch elements.
Device kernel (per core), all matmul compute in fp16 with fp32 PSUM accum:

  1. qkT = (Wqk*s) @ x^T + bias   -> [1536 ch, 1576 tok] channel-major (q,k)
  2. v   = x @ Wv^T + vbias       -> token-major, written into an extended
     layout [v_even|1|...|1|v_odd] per head-pair so the PV matmul emits both
     the transposed attention output and the softmax denominators (rows the
     PV output never reads are left as garbage -- no zero memsets needed).
  3. scores^T per (batch, head-pair): one K=128 matmul per key tile against
     zero-slotted q (both heads at once); exp on ACT (no max subtraction:
     logits are provably tiny), multiplied by host-precomputed
     exp(rel_pos_bias)^T on DVE.
  4. PV: out^T accumulated over key tiles; softmax sums emerge on psum rows
     64/32 via ones columns in the extended v layout; ln(s) on ACT lands
     both sums rows on psum partitions 0/1 of a tiny SBUF tile, a single
     K=2 ones-matmul broadcasts them to all 128 partitions (even-head row
     to partitions 0:64, odd to 64:128), exp(-x) on ACT yields 1/s already
     partition-aligned with the PV output, and a DVE multiply writes the
     proj lhsT layout directly.  (No DRAM bounce / DMA broadcasts.)
  5. proj emitted transposed: y^T[o, t] with out-channels on partitions and
     tokens as the moving dim (no 69-row partition waste), bias as the ACT
     per-partition bias operand, fp16 [DIM, TOK] output transposed back on
     the host; software-pipelined 2 batches behind attention; qkv / v /
     attention interleaved per 2-batch column block.
"""

import sys

for _p in ("/opt/trn_rl_repo",):
    if _p not in sys.path:
        sys.path.insert(0, _p)

import numpy as np

import concourse.bass as bass
import concourse.mybir as mybir
import concourse.tile as tile
from concourse.bass_utils import run_bass_kernel_spmd
from concourse.vector_clock import ScopedClock

# ---------------------------------------------------------------- constants
DIM = 768
NUM_HEADS = 12
WH, WW = 14, 14
N_TOK = WH * WW + 1  # 197
NUM_REL_DIST = (2 * WH - 1) * (2 * WW - 1) + 3  # 732
HEAD_DIM = DIM // NUM_HEADS  # 64
SCALE = HEAD_DIM ** -0.5
B = 64
N_CORES = 8
B_LOC = B // N_CORES  # 8
TOK = B_LOC * N_TOK  # 1576
NPAIR = NUM_HEADS // 2  # 6
KT = [(0, 128), (128, 69)]  # key tiles within a batch
F16 = mybir.dt.float16
F32 = mybir.dt.float32

# ------------------------------------------------- walrus 1-wait workaround
# This walrus build rejects instructions carrying more than one semaphore
# wait ("Too many sync wait commands").  Split extra waits onto same-engine
# NOPs emitted immediately before the instruction during Tile lowering, and
# do the same for the kernel-tail drain's global-clock waits.
_MAXW = 1
_orig_commit_and_lower = tile.TileContext._commit_and_lower


def _patched_commit_and_lower(self, inst, original_block, old_bb_map, bb_to_exit_bb):
    si = inst.sync_info
    if si is not None and si.on_wait is not None and len(si.on_wait) > _MAXW:
        waits = list(si.on_wait)
        for w in waits[:-_MAXW]:
            nop = self.nc.engines[inst.engine].nop(nofuse=True)
            nop.ins.sync_info = mybir.SyncInfo(on_wait=[w], on_update=[])
        inst.sync_info = mybir.SyncInfo(
            on_wait=waits[-_MAXW:], on_update=list(si.on_update or [])
        )
    _orig_commit_and_lower(self, inst, original_block, old_bb_map, bb_to_exit_bb)


def _patched_drain_and_barrier(self, tick_clock, wait_clock):
    nc = self.nc
    probe = nc.sync.nop(nofuse=True)
    wait_clock.add_sem_waits(probe.ins, ScopedClock({None: tick_clock.global_clock}))
    si = probe.ins.sync_info
    waits = list(si.on_wait) if si is not None else []
    if len(waits) > _MAXW:
        probe.ins.sync_info = mybir.SyncInfo(on_wait=waits[:_MAXW], on_update=[])
        for i in range(_MAXW, len(waits), _MAXW):
            extra = nc.sync.nop(nofuse=True)
            extra.ins.sync_info = mybir.SyncInfo(
                on_wait=waits[i : i + _MAXW], on_update=[]
            )
    nc.sync.drain()
    nc.all_engine_barrier()
    assert self.sems is not None
    popped = nc._tile_sem_poison_stack.pop()
    assert popped is self._sem_poison
    nc.clear_and_free_semaphores(list(self.sems.allocated().values()))
    nc.all_engine_barrier()


def _act_recip_lnexp(nc, out, in_):
    # 1/s computed as exp(-ln(s)) on the Scalar engine.  ln and exp live in
    # the same activation table (natural_log_exp_and_others) so this costs
    # no ACT_TABLE_LOAD swaps, unlike the Reciprocal table op; and the DVE
    # InstReciprocal runs at ~6.6 ns/element on one partition (125 us total
    # here).  Inputs are softmax sums in [~50, 4000]: ln/exp tables are
    # accurate to ~1e-4 there, well inside tolerance.
    nc.scalar.activation(out, in_, mybir.ActivationFunctionType.Ln)
    nc.scalar.activation(out, out, mybir.ActivationFunctionType.Exp, scale=-1.0)


def _install_patches():
    tile.TileContext._commit_and_lower = _patched_commit_and_lower
    tile.TileContext._drain_and_barrier = _patched_drain_and_barrier


# ---------------------------------------------------------------- host prep
def _relative_position_index():
    coords = np.stack(np.meshgrid(np.arange(WH), np.arange(WW), indexing="ij"))
    cf = coords.reshape(2, -1)
    rel = cf[:, :, None] - cf[:, None, :]
    rel = rel.transpose(1, 2, 0).astype(np.int64)
    rel[:, :, 0] += WH - 1
    rel[:, :, 1] += WW - 1
    rel[:, :, 0] *= 2 * WW - 1
    idx = np.zeros((N_TOK, N_TOK), dtype=np.int64)
    idx[1:, 1:] = rel.sum(-1)
    idx[0, 0:] = NUM_REL_DIST - 3
    idx[0:, 0] = NUM_REL_DIST - 2
    idx[0, 0] = NUM_REL_DIST - 1
    return idx


def _host_prepare(x, qkv_weight, q_bias, v_bias, rel_pos_bias_table, proj_weight,
                  proj_bias):
    wqk = qkv_weight[: 2 * DIM].astype(np.float32).copy()
    wqk[:DIM] *= SCALE
    wqkT = np.ascontiguousarray(wqk.T).astype(np.float16)  # [768, 1536]
    wvT = np.ascontiguousarray(qkv_weight[2 * DIM :].T).astype(np.float16)
    wprojT = np.ascontiguousarray(proj_weight.T).astype(np.float16)  # [in, out]
    qkb = np.ascontiguousarray(
        (q_bias.astype(np.float32) * SCALE).reshape(6, 128).T
    ).astype(np.float32)  # [128, 6]
    vb = v_bias.astype(np.float16)
    pb = np.ascontiguousarray(
        proj_bias.astype(np.float32).reshape(6, 128).T
    )  # [128, 6]

    idx = _relative_position_index()
    bias_qk = rel_pos_bias_table.astype(np.float32)[idx]  # [q, k, h]
    e = np.exp(bias_qk).transpose(1, 2, 0)  # [k, h, q]
    e = np.ascontiguousarray(e.reshape(N_TOK, NPAIR, 2 * N_TOK)).astype(np.float16)
    rpb0 = np.ascontiguousarray(e[:128])  # [128, 6, 394]
    rpb1 = np.ascontiguousarray(e[128:])  # [69, 6, 394]

    shared = dict(wqkT=wqkT, wvT=wvT, wprojT=wprojT, qkb=qkb, vbias=vb,
                  pbias=pb, rpb0=rpb0, rpb1=rpb1)
    in_maps = []
    for c in range(N_CORES):
        xc = x[c * B_LOC : (c + 1) * B_LOC].reshape(TOK, DIM).astype(np.float16)
        xT = np.ascontiguousarray(xc.T)  # [768, 1576]
        in_maps.append(dict(xT=xT, **shared))
    return in_maps


# ------------------------------------------------------------- device build
def build_nc(phases=4, sub=9):
    _install_patches()
    nc = bass.Bass("TRN2", target_bir_lowering=False, debug=False,
                   num_devices=N_CORES)

    xT = nc.dram_tensor("xT", [DIM, TOK], F16, kind="ExternalInput")
    wqkT = nc.dram_tensor("wqkT", [DIM, 2 * DIM], F16, kind="ExternalInput")
    wvT = nc.dram_tensor("wvT", [DIM, DIM], F16, kind="ExternalInput")
    wprojT = nc.dram_tensor("wprojT", [DIM, DIM], F16, kind="ExternalInput")
    qkb = nc.dram_tensor("qkb", [128, 6], F32, kind="ExternalInput")
    vbias = nc.dram_tensor("vbias", [DIM], F16, kind="ExternalInput")
    pbias = nc.dram_tensor("pbias", [128, 6], F32, kind="ExternalInput")
    rpb0 = nc.dram_tensor("rpb0", [128, NPAIR, 2 * N_TOK], F16, kind="ExternalInput")
    rpb1 = nc.dram_tensor("rpb1", [69, NPAIR, 2 * N_TOK], F16, kind="ExternalInput")
    # output is the transposed projection y^T [DIM, TOK] in fp16; the host
    # transposes back and upcasts after gathering.
    out = nc.dram_tensor("out", [DIM, TOK], F16, kind="ExternalOutput")

    def bcast_ap(handle, n):
        ap = handle.ap()
        return bass.AP(tensor=ap.tensor, offset=ap.offset,
                       ap=[[0, 128]] + list(ap.ap))

    with tile.TileContext(nc) as tc:
        with (
            tc.tile_pool(name="const", bufs=1) as const,
            tc.tile_pool(name="exp", bufs=20) as exp_pool,
            tc.tile_pool(name="attn", bufs=8) as attn_pool,
            tc.tile_pool(name="outp", bufs=6) as out_pool,
            tc.tile_pool(name="rc", bufs=3) as rc_pool,
            tc.tile_pool(name="psA", bufs=3, space="PSUM") as psum,
            tc.tile_pool(name="psB", bufs=2, space="PSUM") as psum_pv,
        ):
            # ---- constants into SBUF
            # Startup loads chunked per consumer (wqkT per m-slice on sync,
            # xT per column block on gpsimd) so the first qkv chunk lands
            # in a few us instead of waiting for the full 9MB.
            NCH = 394
            xT_sb = const.tile([128, 6, TOK], F16, tag="xT")
            xT_r = xT.ap().rearrange("(a p) n -> p a n", p=128)
            wqkT_sb = const.tile([128, 6, 2 * DIM], F16, tag="wqkT")
            wqk_r = wqkT.ap().rearrange("(a p) n -> p a n", p=128)
            # first column block per-k so the k=0 accumulation matmul can
            # start as soon as its 100KB slice lands (~1.5us)
            for k in range(6):
                nc.gpsimd.dma_start(
                    xT_sb[:, k, 0:NCH], xT_r[:, k, 0:NCH])
            for nch in range(1, 4):
                nc.gpsimd.dma_start(
                    xT_sb[:, :, nch * NCH : (nch + 1) * NCH],
                    xT_r[:, :, nch * NCH : (nch + 1) * NCH])
                if nch == 1:
                    vb_sb = const.tile([128, DIM], F16, tag="vb")
                    nc.gpsimd.dma_start(vb_sb[:], bcast_ap(vbias, DIM))
            wvT_sb = const.tile([128, 6, DIM], F16, tag="wvT")
            qkb_sb = const.tile([128, 6], F32, tag="qkb")
            for m in range(12):
                nc.sync.dma_start(
                    wqkT_sb[:, :, m * 128 : (m + 1) * 128],
                    wqk_r[:, :, m * 128 : (m + 1) * 128])
                if m == 0:
                    nc.sync.dma_start(qkb_sb[:], qkb.ap())
                if m == 5:
                    nc.sync.dma_start(
                        wvT_sb[:],
                        wvT.ap().rearrange("(a p) n -> p a n", p=128))
            # rpb is needed by the first attention (~20us in, zippered into
            # the second column block); proj weights/bias only ~40us in.
            rpb0_sb = const.tile([128, NPAIR, 2 * N_TOK], F16, tag="rpb0")
            nc.sync.dma_start(rpb0_sb[:], rpb0.ap())
            rpb1_sb = const.tile([69, NPAIR, 2 * N_TOK], F16, tag="rpb1")
            nc.sync.dma_start(rpb1_sb[:], rpb1.ap())
            wprojT_sb = const.tile([128, 6, DIM], F16, tag="wprojT")
            nc.sync.dma_start(
                wprojT_sb[:], wprojT.ap().rearrange("(a p) n -> p a n", p=128))
            pb_sb = const.tile([128, 6], F32, tag="pb")
            nc.sync.dma_start(pb_sb[:], pbias.ap())

            kT_sb = const.tile([128, 6, TOK], F16, tag="kT")
            # q in zero-padded head slots: slot (c, j) holds head 2c+j on
            # partitions 64j:64j+64, zeros elsewhere, so QK^T runs as a
            # plain K=128 matmul against the packed k chunk.  (Row-group
            # packed K=64 matmul pairs crash this runtime.)
            qz_sb = const.tile([128, 6, 2, TOK], F16, tag="qz")
            nc.gpsimd.memset(qz_sb[0:64, :, 1, :], 0.0)
            nc.gpsimd.memset(qz_sb[64:128, :, 0, :], 0.0)
            # v extended layout per (token-tile, pair):
            #   [0:64]=v_even [64]=1 | odd block (65+): [32]=1 [64:128]=v_odd
            #   -> odd sums land on psum row 32.  Slots the PV output never
            #   reads (j0 rows 65:128 except the sums row feed, j1 rows
            #   0:64 except row 32) stay uninitialized garbage.
            vext_sb = const.tile([128, 2 * B_LOC, NPAIR, 193], F16, tag="vext")
            nc.vector.memset(vext_sb[:, :, :, 64:65], 1.0)
            nc.vector.memset(vext_sb[:, :, :, 97:98], 1.0)
            # Broadcast weights for the softmax-denominator spread: engine
            # partition offsets must be 32-aligned, so the two ln(s) rows
            # live on partitions 0 (even heads) and 32 (odd heads) and the
            # matmul contracts K=33.  ones row 0 -> M rows 0:64, ones row
            # 32 -> M rows 64:128; partitions 1..31 are zeroed once (both
            # here and in lns_sb) so the dead rows contribute exact zeros.
            ones_sb = const.tile([33, 2, 64], F16, tag="ones")
            nc.vector.memset(ones_sb[0:33, :, :], 0.0)
            nc.vector.memset(ones_sb[0:1, 0, :], 1.0)
            nc.vector.memset(ones_sb[32:33, 1, :], 1.0)
            # ln(s) landing tile, manually double-buffered by group parity.
            lns_sb = const.tile([33, 2, 2, N_TOK], F16, tag="lns")
            nc.vector.memset(lns_sb[0:33, :, :, :], 0.0)

            # ---- phase 3: attention per (batch, pair-group of 2 head-pairs)
            # Transposed proj: y^T[o, t] with out-channels on partitions and
            # tokens moving (no partition waste from 197-token batches).
            # Emission is software-pipelined: proj m-chunks of batch b-2 are
            # spread through batch b's scores phase as PE filler while ACT
            # chews the exps, and the softmax-normalize of group g (bcast
            # matmul -> exp(-x) -> DVE muls) is deferred until after group
            # g+1's PV so the PE never waits on the Ln chain.
            proj_fifo = []

            def emit_proj_chunk(b, attn_sb, m):
                q0 = b * N_TOK
                # alternate between the dedicated pj bank and the shared ps
                # pool so back-to-back chunks (tail flush) pipeline 2-deep
                # instead of serializing MM -> drain -> MM on one bank.
                if m % 2:
                    ps = psum.tile([128, N_TOK], F32, tag="ps")
                else:
                    ps = psum.tile([128, N_TOK], F32, tag="pj", bufs=1)
                for k in range(6):
                    nc.tensor.matmul(
                        ps[:],
                        lhsT=wprojT_sb[:, k, m * 128 : (m + 1) * 128],
                        rhs=attn_sb[:, k, :],
                        start=(k == 0), stop=(k == 5),
                    )
                osb = out_pool.tile([128, N_TOK], F16, tag="osb")
                nc.vector.tensor_add(
                    out=osb[:], in0=ps[:],
                    in1=pb_sb[:, m : m + 1].to_broadcast([128, N_TOK]))
                oq = nc.sync if (b + m) % 2 else nc.gpsimd
                oq.dma_start(
                    out.ap()[m * 128 : (m + 1) * 128, q0 : q0 + N_TOK],
                    osb[:])

            def emit_proj_fill(n):
                for _ in range(min(n, len(proj_fifo))):
                    emit_proj_chunk(*proj_fifo.pop(0))

            norm_fifo = []

            def emit_norm():
                b, g, gpar, attn_sb, pvg = norm_fifo.pop(0)
                bcp = psum.tile([128, 2, N_TOK], F32, tag="ps")
                nc.tensor.matmul(
                    bcp[:, :, :], lhsT=ones_sb[0:33, :, :],
                    rhs=lns_sb[0:33, gpar, :, :], start=True, stop=True)
                rbc = rc_pool.tile([128, 2, N_TOK], F32, tag="rbc")
                nc.scalar.activation(
                    rbc[:], bcp[:],
                    mybir.ActivationFunctionType.Exp, scale=-1.0)
                for pig in (0, 1):
                    c = 2 * g + pig
                    nc.vector.tensor_mul(
                        attn_sb[0:64, c, :], pvg[0:64, pig, 0, 0:N_TOK],
                        rbc[0:64, pig, :])
                    nc.vector.tensor_mul(
                        attn_sb[64:128, c, :], pvg[64:128, pig, 1, 0:N_TOK],
                        rbc[64:128, pig, :])
                # the batch's attn_sb is fully written only once its last
                # group's normalize is EMITTED -- only then may proj chunks
                # (which read all 6 pair-slices) enter the fill queue, or
                # they would be emitted before their producers and the
                # dependency tracker would never order them.
                if g == NPAIR // 2 - 1:
                    proj_fifo.extend((b, attn_sb, m) for m in range(6))

            # ---- phases 1-3, zippered: each column block's qkv/v GEMM
            # chunks (PE-heavy, ACT-light) are interleaved with the
            # PREVIOUS block's attention closures (ACT-heavy) so neither
            # engine goes idle for a whole phase.
            def mk_qkv_chunk(nch, m):
                def run():
                    ps = psum.tile([128, NCH], F32, tag="ps")
                    for k in range(6):
                        nc.tensor.matmul(
                            ps[:],
                            lhsT=wqkT_sb[:, k, m * 128 : (m + 1) * 128],
                            rhs=xT_sb[:, k, nch * NCH : (nch + 1) * NCH],
                            start=(k == 0), stop=(k == 5),
                        )
                    cols = slice(nch * NCH, (nch + 1) * NCH)
                    if m < 6:
                        # bias-add + f16 downcast drain on DVE (ACT is the
                        # busier engine); qkb broadcast along the free dim.
                        nc.vector.tensor_add(
                            out=qz_sb[0:64, m, 0, cols], in0=ps[0:64],
                            in1=qkb_sb[0:64, m : m + 1].to_broadcast([64, NCH]),
                        )
                        nc.vector.tensor_add(
                            out=qz_sb[64:128, m, 1, cols], in0=ps[64:128],
                            in1=qkb_sb[64:128, m : m + 1].to_broadcast([64, NCH]),
                        )
                    else:
                        nc.scalar.activation(
                            kT_sb[:, m - 6, cols], ps[:],
                            mybir.ActivationFunctionType.Copy)
                return run

            def mk_v_chunk(bt, ncb):
                def run():
                    b, t = divmod(bt, 2)
                    tbase, tsz = KT[t]
                    col0 = b * N_TOK + tbase
                    ps = psum.tile([128, 384], F32, tag="ps")
                    for k in range(6):
                        nc.tensor.matmul(
                            ps[:tsz],
                            lhsT=xT_sb[:, k, col0 : col0 + tsz],
                            rhs=wvT_sb[:, k, ncb * 384 : (ncb + 1) * 384],
                            start=(k == 0), stop=(k == 5),
                        )
                    src = ps[:tsz].rearrange("p (c j d) -> p c j d", c=3, j=2)
                    vbv = vb_sb[:tsz, ncb * 384 : (ncb + 1) * 384].rearrange(
                        "p (c j d) -> p c j d", c=3, j=2)
                    # both j-halves in one DVE op: the two 64-wide v slots
                    # sit at free offsets 0 and 129 of the vext row (stride
                    # 129 exactly covers [0:64] and [129:193]).
                    pear = vext_sb[:tsz, bt, 3 * ncb : 3 * ncb + 3, :]
                    dst = bass.AP(
                        tensor=pear.tensor, offset=pear.offset,
                        ap=list(pear.ap)[:-1] + [[129, 2], [1, 64]])
                    nc.vector.tensor_add(out=dst, in0=src, in1=vbv)
                return run

            def mk_scores(b, g, ets_all):
                def run():
                    q0 = b * N_TOK
                    for pig in (0, 1):
                        c = 2 * g + pig
                        ets = []
                        for t, (kbase, ksz) in enumerate(KT):
                            kcol = q0 + kbase
                            ps = psum.tile([128, 2 * N_TOK], F32, tag="ps")
                            nc.tensor.matmul(
                                ps[:ksz, :].rearrange("p (j q) -> p j q", j=2),
                                lhsT=kT_sb[:, c, kcol : kcol + ksz],
                                rhs=qz_sb[:, c, :, q0 : q0 + N_TOK],
                                start=True, stop=True,
                            )
                            et = exp_pool.tile([128, 2 * N_TOK], F16, tag="exp")
                            nc.scalar.activation(
                                et[:ksz], ps[:ksz], mybir.ActivationFunctionType.Exp)
                            rp = rpb0_sb if t == 0 else rpb1_sb
                            nc.vector.tensor_mul(et[:ksz], et[:ksz], rp[:ksz, c, :])
                            ets.append((et, ksz))
                        ets_all[(g, pig)] = ets
                    emit_proj_fill(1)
                return run

            def mk_pv(b, g, attn_sb, ets_all):
                def run():
                    gpar = (b * 3 + g) % 2
                    # PV psum for 2 pairs; 256-stride keeps each matmul
                    # region inside one PSUM bank.
                    pvg = psum_pv.tile([128, 2, 2, 256], F32, tag="pvg")
                    for pig in (0, 1):
                        c = 2 * g + pig
                        for j in (0, 1):
                            outap = pvg[:, pig, j, 0:N_TOK]
                            lo, hi = (0, 128) if j == 0 else (65, 193)
                            for t, (et, ksz) in enumerate(ets_all[(g, pig)]):
                                nc.tensor.matmul(
                                    outap,
                                    lhsT=vext_sb[:ksz, 2 * b + t, c, lo:hi],
                                    rhs=et[:ksz, j * N_TOK : (j + 1) * N_TOK],
                                    start=(t == 0), stop=(t == 1),
                                )
                    # softmax denominators: psum row 64 (even heads, j=0)
                    # and row 32 (odd heads, j=1).  ln(s) in f16 (same ACT
                    # table as Exp) lands both pigs' rows on partitions
                    # 0/32 of lns_sb; the deferred K=33 ones-matmul
                    # broadcasts 1/s partition-aligned with the PV output.
                    nc.scalar.activation(
                        lns_sb[0:1, gpar, :, :], pvg[64:65, :, 0, 0:N_TOK],
                        mybir.ActivationFunctionType.Ln)
                    nc.scalar.activation(
                        lns_sb[32:33, gpar, :, :], pvg[32:33, :, 1, 0:N_TOK],
                        mybir.ActivationFunctionType.Ln)
                    norm_fifo.append((b, g, gpar, attn_sb, pvg))
                    # the last two batches have no later work to hide norm
                    # latency behind -- emit eagerly so their proj chunks
                    # unlock as early as possible.
                    while len(norm_fifo) > (1 if b < 6 else 0):
                        emit_norm()
                    emit_proj_fill(1)
                return run

            def mk_attn_closures(b):
                attn_sb = attn_pool.tile([128, 6, N_TOK], F16, tag="attn")
                ets_all = {}
                cls = []
                for g in range(NPAIR // 2):
                    cls.append(mk_scores(b, g, ets_all))
                for g in range(NPAIR // 2):
                    cls.append(mk_pv(b, g, attn_sb, ets_all))
                return cls

            for nch in range(4):
                qc = [mk_qkv_chunk(nch, m) for m in range(12)]
                vc = [mk_v_chunk(bt, ncb)
                      for bt in range(4 * nch, 4 * nch + 4)
                      for ncb in range(2)]
                # v interleaves with the later qkv chunks (it only needs
                # xT + wvT, both early arrivals) so the PE has independent
                # work to absorb weight-DMA jitter.  The last block also
                # zippers its OWN batches' attention (6,7) in behind the
                # chunks they consume; each closure carries the minimum
                # number of gemm chunks that must be EMITTED first (the
                # dependency tracker only orders reads emitted after their
                # producers -- an early emission would be a silent race).
                if nch < 3:
                    gemm = qc[:6]
                    for i in range(8):
                        if i < 6:
                            gemm.append(qc[6 + i])
                        gemm.append(vc[i])
                else:
                    gemm = qc + [vc[0], vc[2], vc[1], vc[3],
                                 vc[4], vc[6], vc[5], vc[7]]
                attn = []
                if phases >= 3 and nch >= 1:
                    attn = [(0, cl) for cl in
                            mk_attn_closures(2 * (nch - 1))
                            + mk_attn_closures(2 * nch - 1)]
                    if nch == 3:
                        # closure order per batch: S0 S1 S2 P0 P1 P2
                        attn += list(zip(
                            [8, 10, 12, 14, 16, 16],
                            mk_attn_closures(6)))
                        attn += list(zip(
                            [8, 10, 12, 18, 20, 20],
                            mk_attn_closures(7)))
                # zipper: spread the attention closures through the gemm
                # chunks (gemm leads; both lists drain fully).
                na, ng = len(attn), len(gemm)
                ai = 0
                for gi, gcl in enumerate(gemm):
                    gcl()
                    while (ai < na and attn[ai][0] <= gi + 1
                           and ai * ng < na * (gi + 1)):
                        attn[ai][1]()
                        ai += 1
                while ai < na:
                    attn[ai][1]()
                    ai += 1
            while norm_fifo:
                emit_norm()
            if phases >= 4:
                emit_proj_fill(len(proj_fifo))
            if phases < 4:
                # debug: dump some qkT into out so the output is written
                dbg = out_pool.tile([128, TOK], F16, tag="dbg")
                nc.scalar.activation(dbg[:], kT_sb[:, 0, :],
                                     mybir.ActivationFunctionType.Copy)
                for r in range(0, DIM, 128):
                    nc.sync.dma_start(out.ap()[r : r + 128, :], dbg[:])
    return nc


_NC_CACHE = None


def _get_nc():
    global _NC_CACHE
    if _NC_CACHE is None:
        _NC_CACHE = build_nc()
    return _NC_CACHE


def _execute(inputs, trace=False):
    in_maps = _host_prepare(**inputs)
    nc = _get_nc()
    res = run_bass_kernel_spmd(nc, in_maps, core_ids=list(range(N_CORES)),
                               trace=trace)
    outs = [
        np.ascontiguousarray(res.results[c]["out"].T)
        .astype(np.float32)
        .reshape(B_LOC, N_TOK, DIM)
        for c in range(N_CORES)
    ]
    return np.concatenate(outs, axis=0), res


def kernel(**inputs) -> np.ndarray:
    out, _ = _execute(inputs, trace=False)
    return out



# revision 45
# speedup vs baseline: 1.0129x; 1.0129x over previous
"""BEiT-style windowed attention block on 8 TRN2 NeuronCores.

Data-parallel over batch: each core handles 8 of the 64 batch elements.
Device kernel (per core), all matmul compute in fp16 with fp32 PSUM accum:

  1. qkT = (Wqk*s) @ x^T + bias   -> [1536 ch, 1576 tok] channel-major (q,k)
  2. v   = x @ Wv^T + vbias       -> token-major, written into an extended
     layout [v_even|1|...|1|v_odd] per head-pair so the PV matmul emits both
     the transposed attention output and the softmax denominators (rows the
     PV output never reads are left as garbage -- no zero memsets needed).
  3. scores^T per (batch, head-pair): one K=128 matmul per key tile against
     zero-slotted q (both heads at once); exp on ACT (no max subtraction:
     logits are provably tiny), multiplied by host-precomputed
     exp(rel_pos_bias)^T on DVE.
  4. PV: out^T accumulated over key tiles; softmax sums emerge on psum rows
     64/32 via ones columns in the extended v layout; ln(s) on ACT lands
     both sums rows on psum partitions 0/1 of a tiny SBUF tile, a single
     K=2 ones-matmul broadcasts them to all 128 partitions (even-head row
     to partitions 0:64, odd to 64:128), exp(-x) on ACT yields 1/s already
     partition-aligned with the PV output, and a DVE multiply writes the
     proj lhsT layout directly.  (No DRAM bounce / DMA broadcasts.)
  5. proj emitted transposed: y^T[o, t] with out-channels on partitions and
     tokens as the moving dim (no 69-row partition waste), bias as the ACT
     per-partition bias operand, fp16 [DIM, TOK] output transposed back on
     the host; software-pipelined 2 batches behind attention; qkv / v /
     attention interleaved per 2-batch column block.
"""

import sys

for _p in ("/opt/trn_rl_repo",):
    if _p not in sys.path:
        sys.path.insert(0, _p)

import numpy as np

import concourse.bass as bass
import concourse.mybir as mybir
import concourse.tile as tile
from concourse.bass_utils import run_bass_kernel_spmd
from concourse.vector_clock import ScopedClock

# ---------------------------------------------------------------- constants
DIM = 768
NUM_HEADS = 12
WH, WW = 14, 14
N_TOK = WH * WW + 1  # 197
NUM_REL_DIST = (2 * WH - 1) * (2 * WW - 1) + 3  # 732
HEAD_DIM = DIM // NUM_HEADS  # 64
SCALE = HEAD_DIM ** -0.5
B = 64
N_CORES = 8
B_LOC = B // N_CORES  # 8
TOK = B_LOC * N_TOK  # 1576
NPAIR = NUM_HEADS // 2  # 6
KT = [(0, 128), (128, 69)]  # key tiles within a batch
F16 = mybir.dt.float16
F32 = mybir.dt.float32

# ------------------------------------------------- walrus 1-wait workaround
# This walrus build rejects instructions carrying more than one semaphore
# wait ("Too many sync wait commands").  Split extra waits onto same-engine
# NOPs emitted immediately before the instruction during Tile lowering, and
# do the same for the kernel-tail drain's global-clock waits.
_MAXW = 1
_orig_commit_and_lower = tile.TileContext._commit_and_lower


def _patched_commit_and_lower(self, inst, original_block, old_bb_map, bb_to_exit_bb):
    si = inst.sync_info
    if si is not None and si.on_wait is not None and len(si.on_wait) > _MAXW:
        waits = list(si.on_wait)
        for w in waits[:-_MAXW]:
            nop = self.nc.engines[inst.engine].nop(nofuse=True)
            nop.ins.sync_info = mybir.SyncInfo(on_wait=[w], on_update=[])
        inst.sync_info = mybir.SyncInfo(
            on_wait=waits[-_MAXW:], on_update=list(si.on_update or [])
        )
    _orig_commit_and_lower(self, inst, original_block, old_bb_map, bb_to_exit_bb)


def _patched_drain_and_barrier(self, tick_clock, wait_clock):
    nc = self.nc
    probe = nc.sync.nop(nofuse=True)
    wait_clock.add_sem_waits(probe.ins, ScopedClock({None: tick_clock.global_clock}))
    si = probe.ins.sync_info
    waits = list(si.on_wait) if si is not None else []
    if len(waits) > _MAXW:
        probe.ins.sync_info = mybir.SyncInfo(on_wait=waits[:_MAXW], on_update=[])
        for i in range(_MAXW, len(waits), _MAXW):
            extra = nc.sync.nop(nofuse=True)
            extra.ins.sync_info = mybir.SyncInfo(
                on_wait=waits[i : i + _MAXW], on_update=[]
            )
    nc.sync.drain()
    nc.all_engine_barrier()
    assert self.sems is not None
    popped = nc._tile_sem_poison_stack.pop()
    assert popped is self._sem_poison
    nc.clear_and_free_semaphores(list(self.sems.allocated().values()))
    nc.all_engine_barrier()


def _act_recip_lnexp(nc, out, in_):
    # 1/s computed as exp(-ln(s)) on the Scalar engine.  ln and exp live in
    # the same activation table (natural_log_exp_and_others) so this costs
    # no ACT_TABLE_LOAD swaps, unlike the Reciprocal table op; and the DVE
    # InstReciprocal runs at ~6.6 ns/element on one partition (125 us total
    # here).  Inputs are softmax sums in [~50, 4000]: ln/exp tables are
    # accurate to ~1e-4 there, well inside tolerance.
    nc.scalar.activation(out, in_, mybir.ActivationFunctionType.Ln)
    nc.scalar.activation(out, out, mybir.ActivationFunctionType.Exp, scale=-1.0)


def _install_patches():
    tile.TileContext._commit_and_lower = _patched_commit_and_lower
    tile.TileContext._drain_and_barrier = _patched_drain_and_barrier


# ---------------------------------------------------------------- host prep
def _relative_position_index():
    coords = np.stack(np.meshgrid(np.arange(WH), np.arange(WW), indexing="ij"))
    cf = coords.reshape(2, -1)
    rel = cf[:, :, None] - cf[:, None, :]
    rel = rel.transpose(1, 2, 0).astype(np.int64)
    rel[:, :, 0] += WH - 1
    rel[:, :, 1] += WW - 1
    rel[:, :, 0] *= 2 * WW - 1
    idx = np.zeros((N_TOK, N_TOK), dtype=np.int64)
    idx[1:, 1:] = rel.sum(-1)
    idx[0, 0:] = NUM_REL_DIST - 3
    idx[0:, 0] = NUM_REL_DIST - 2
    idx[0, 0] = NUM_REL_DIST - 1
    return idx


def _host_prepare(x, qkv_weight, q_bias, v_bias, rel_pos_bias_table, proj_weight,
                  proj_bias):
    wqk = qkv_weight[: 2 * DIM].astype(np.float32).copy()
    wqk[:DIM] *= SCALE
    wqkT = np.ascontiguousarray(wqk.T).astype(np.float16)  # [768, 1536]
    wvT = np.ascontiguousarray(qkv_weight[2 * DIM :].T).astype(np.float16)
    wprojT = np.ascontiguousarray(proj_weight.T).astype(np.float16)  # [in, out]
    qkb = np.ascontiguousarray(
        (q_bias.astype(np.float32) * SCALE).reshape(6, 128).T
    ).astype(np.float32)  # [128, 6]
    vb = v_bias.astype(np.float16)
    pb = np.ascontiguousarray(
        proj_bias.astype(np.float32).reshape(6, 128).T
    )  # [128, 6]

    idx = _relative_position_index()
    bias_qk = rel_pos_bias_table.astype(np.float32)[idx]  # [q, k, h]
    e = np.exp(bias_qk).transpose(1, 2, 0)  # [k, h, q]
    e = np.ascontiguousarray(e.reshape(N_TOK, NPAIR, 2 * N_TOK)).astype(np.float16)
    rpb0 = np.ascontiguousarray(e[:128])  # [128, 6, 394]
    rpb1 = np.ascontiguousarray(e[128:])  # [69, 6, 394]

    shared = dict(wqkT=wqkT, wvT=wvT, wprojT=wprojT, qkb=qkb, vbias=vb,
                  pbias=pb, rpb0=rpb0, rpb1=rpb1)
    in_maps = []
    for c in range(N_CORES):
        xc = x[c * B_LOC : (c + 1) * B_LOC].reshape(TOK, DIM).astype(np.float16)
        xT = np.ascontiguousarray(xc.T)  # [768, 1576]
        in_maps.append(dict(xT=xT, **shared))
    return in_maps


# ------------------------------------------------------------- device build
def build_nc(phases=4, sub=9):
    _install_patches()
    nc = bass.Bass("TRN2", target_bir_lowering=False, debug=False,
                   num_devices=N_CORES)

    xT = nc.dram_tensor("xT", [DIM, TOK], F16, kind="ExternalInput")
    wqkT = nc.dram_tensor("wqkT", [DIM, 2 * DIM], F16, kind="ExternalInput")
    wvT = nc.dram_tensor("wvT", [DIM, DIM], F16, kind="ExternalInput")
    wprojT = nc.dram_tensor("wprojT", [DIM, DIM], F16, kind="ExternalInput")
    qkb = nc.dram_tensor("qkb", [128, 6], F32, kind="ExternalInput")
    vbias = nc.dram_tensor("vbias", [DIM], F16, kind="ExternalInput")
    pbias = nc.dram_tensor("pbias", [128, 6], F32, kind="ExternalInput")
    rpb0 = nc.dram_tensor("rpb0", [128, NPAIR, 2 * N_TOK], F16, kind="ExternalInput")
    rpb1 = nc.dram_tensor("rpb1", [69, NPAIR, 2 * N_TOK], F16, kind="ExternalInput")
    # output is the transposed projection y^T [DIM, TOK] in fp16; the host
    # transposes back and upcasts after gathering.
    out = nc.dram_tensor("out", [DIM, TOK], F16, kind="ExternalOutput")

    def bcast_ap(handle, n):
        ap = handle.ap()
        return bass.AP(tensor=ap.tensor, offset=ap.offset,
                       ap=[[0, 128]] + list(ap.ap))

    with tile.TileContext(nc) as tc:
        with (
            tc.tile_pool(name="const", bufs=1) as const,
            tc.tile_pool(name="exp", bufs=20) as exp_pool,
            tc.tile_pool(name="attn", bufs=8) as attn_pool,
            tc.tile_pool(name="outp", bufs=6) as out_pool,
            tc.tile_pool(name="rc", bufs=3) as rc_pool,
            tc.tile_pool(name="psA", bufs=3, space="PSUM") as psum,
            tc.tile_pool(name="psB", bufs=2, space="PSUM") as psum_pv,
        ):
            # ---- constants into SBUF
            # Startup loads chunked per consumer (wqkT per m-slice on sync,
            # xT per column block on gpsimd) so the first qkv chunk lands
            # in a few us instead of waiting for the full 9MB.
            NCH = 394
            xT_sb = const.tile([128, 6, TOK], F16, tag="xT")
            xT_r = xT.ap().rearrange("(a p) n -> p a n", p=128)
            wqkT_sb = const.tile([128, 6, 2 * DIM], F16, tag="wqkT")
            wqk_r = wqkT.ap().rearrange("(a p) n -> p a n", p=128)
            # first column block per-k so the k=0 accumulation matmul can
            # start as soon as its 100KB slice lands (~1.5us)
            for k in range(6):
                nc.gpsimd.dma_start(
                    xT_sb[:, k, 0:NCH], xT_r[:, k, 0:NCH])
            for nch in range(1, 4):
                nc.gpsimd.dma_start(
                    xT_sb[:, :, nch * NCH : (nch + 1) * NCH],
                    xT_r[:, :, nch * NCH : (nch + 1) * NCH])
                if nch == 1:
                    vb_sb = const.tile([128, DIM], F16, tag="vb")
                    nc.gpsimd.dma_start(vb_sb[:], bcast_ap(vbias, DIM))
            wvT_sb = const.tile([128, 6, DIM], F16, tag="wvT")
            qkb_sb = const.tile([128, 6], F32, tag="qkb")
            for m in range(12):
                nc.sync.dma_start(
                    wqkT_sb[:, :, m * 128 : (m + 1) * 128],
                    wqk_r[:, :, m * 128 : (m + 1) * 128])
                if m == 0:
                    nc.sync.dma_start(qkb_sb[:], qkb.ap())
                if m == 5:
                    nc.sync.dma_start(
                        wvT_sb[:],
                        wvT.ap().rearrange("(a p) n -> p a n", p=128))
            # rpb is needed by the first attention (~20us in, zippered into
            # the second column block); proj weights/bias only ~40us in.
            rpb0_sb = const.tile([128, NPAIR, 2 * N_TOK], F16, tag="rpb0")
            nc.sync.dma_start(rpb0_sb[:], rpb0.ap())
            rpb1_sb = const.tile([69, NPAIR, 2 * N_TOK], F16, tag="rpb1")
            nc.sync.dma_start(rpb1_sb[:], rpb1.ap())
            wprojT_sb = const.tile([128, 6, DIM], F16, tag="wprojT")
            nc.sync.dma_start(
                wprojT_sb[:], wprojT.ap().rearrange("(a p) n -> p a n", p=128))
            pb_sb = const.tile([128, 6], F32, tag="pb")
            nc.sync.dma_start(pb_sb[:], pbias.ap())

            kT_sb = const.tile([128, 6, TOK], F16, tag="kT")
            # q in zero-padded head slots: slot (c, j) holds head 2c+j on
            # partitions 64j:64j+64, zeros elsewhere, so QK^T runs as a
            # plain K=128 matmul against the packed k chunk.  (Row-group
            # packed K=64 matmul pairs crash this runtime.)
            qz_sb = const.tile([128, 6, 2, TOK], F16, tag="qz")
            nc.gpsimd.memset(qz_sb[0:64, :, 1, :], 0.0)
            nc.gpsimd.memset(qz_sb[64:128, :, 0, :], 0.0)
            # v extended layout per (token-tile, pair):
            #   [0:64]=v_even [64]=1 | odd block (65+): [32]=1 [64:128]=v_odd
            #   -> odd sums land on psum row 32.  Slots the PV output never
            #   reads (j0 rows 65:128 except the sums row feed, j1 rows
            #   0:64 except row 32) stay uninitialized garbage.
            vext_sb = const.tile([128, 2 * B_LOC, NPAIR, 193], F16, tag="vext")
            nc.vector.memset(vext_sb[:, :, :, 64:65], 1.0)
            nc.vector.memset(vext_sb[:, :, :, 97:98], 1.0)
            # Broadcast weights for the softmax-denominator spread: engine
            # partition offsets must be 32-aligned, so the two ln(s) rows
            # live on partitions 0 (even heads) and 32 (odd heads) and the
            # matmul contracts K=33.  ones row 0 -> M rows 0:64, ones row
            # 32 -> M rows 64:128; partitions 1..31 are zeroed once (both
            # here and in lns_sb) so the dead rows contribute exact zeros.
            ones_sb = const.tile([33, 2, 64], F16, tag="ones")
            nc.vector.memset(ones_sb[0:33, :, :], 0.0)
            nc.vector.memset(ones_sb[0:1, 0, :], 1.0)
            nc.vector.memset(ones_sb[32:33, 1, :], 1.0)
            # ln(s) landing tile, manually double-buffered by group parity.
            lns_sb = const.tile([33, 2, 2, N_TOK], F16, tag="lns")
            nc.vector.memset(lns_sb[0:33, :, :, :], 0.0)

            # ---- phase 3: attention per (batch, pair-group of 2 head-pairs)
            # Transposed proj: y^T[o, t] with out-channels on partitions and
            # tokens moving (no partition waste from 197-token batches).
            # Emission is software-pipelined: proj m-chunks of batch b-2 are
            # spread through batch b's scores phase as PE filler while ACT
            # chews the exps, and the softmax-normalize of group g (bcast
            # matmul -> exp(-x) -> DVE muls) is deferred until after group
            # g+1's PV so the PE never waits on the Ln chain.
            proj_fifo = []

            def emit_proj_chunk(b, attn_sb, m):
                q0 = b * N_TOK
                # alternate between the dedicated pj bank and the shared ps
                # pool so back-to-back chunks (tail flush) pipeline 2-deep
                # instead of serializing MM -> drain -> MM on one bank.
                if m % 2:
                    ps = psum.tile([128, N_TOK], F32, tag="ps")
                else:
                    ps = psum.tile([128, N_TOK], F32, tag="pj", bufs=1)
                for k in range(6):
                    nc.tensor.matmul(
                        ps[:],
                        lhsT=wprojT_sb[:, k, m * 128 : (m + 1) * 128],
                        rhs=attn_sb[:, k, :],
                        start=(k == 0), stop=(k == 5),
                    )
                osb = out_pool.tile([128, N_TOK], F16, tag="osb")
                nc.vector.tensor_add(
                    out=osb[:], in0=ps[:],
                    in1=pb_sb[:, m : m + 1].to_broadcast([128, N_TOK]))
                oq = nc.sync if (b + m) % 2 else nc.gpsimd
                oq.dma_start(
                    out.ap()[m * 128 : (m + 1) * 128, q0 : q0 + N_TOK],
                    osb[:])

            def emit_proj_fill(n):
                for _ in range(min(n, len(proj_fifo))):
                    emit_proj_chunk(*proj_fifo.pop(0))

            norm_fifo = []

            def emit_norm():
                b, g, gpar, attn_sb, pvg = norm_fifo.pop(0)
                bcp = psum.tile([128, 2, N_TOK], F32, tag="ps")
                nc.tensor.matmul(
                    bcp[:, :, :], lhsT=ones_sb[0:33, :, :],
                    rhs=lns_sb[0:33, gpar, :, :], start=True, stop=True)
                rbc = rc_pool.tile([128, 2, N_TOK], F32, tag="rbc")
                nc.scalar.activation(
                    rbc[:], bcp[:],
                    mybir.ActivationFunctionType.Exp, scale=-1.0)
                for pig in (0, 1):
                    c = 2 * g + pig
                    nc.vector.tensor_mul(
                        attn_sb[0:64, c, :], pvg[0:64, pig, 0, 0:N_TOK],
                        rbc[0:64, pig, :])
                    nc.vector.tensor_mul(
                        attn_sb[64:128, c, :], pvg[64:128, pig, 1, 0:N_TOK],
                        rbc[64:128, pig, :])
                # the batch's attn_sb is fully written only once its last
                # group's normalize is EMITTED -- only then may proj chunks
                # (which read all 6 pair-slices) enter the fill queue, or
                # they would be emitted before their producers and the
                # dependency tracker would never order them.
                if g == NPAIR // 2 - 1:
                    proj_fifo.extend((b, attn_sb, m) for m in range(6))

            # ---- phases 1-3, zippered: each column block's qkv/v GEMM
            # chunks (PE-heavy, ACT-light) are interleaved with the
            # PREVIOUS block's attention closures (ACT-heavy) so neither
            # engine goes idle for a whole phase.
            def mk_qkv_chunk(nch, m):
                def run():
                    ps = psum.tile([128, NCH], F32, tag="ps")
                    for k in range(6):
                        nc.tensor.matmul(
                            ps[:],
                            lhsT=wqkT_sb[:, k, m * 128 : (m + 1) * 128],
                            rhs=xT_sb[:, k, nch * NCH : (nch + 1) * NCH],
                            start=(k == 0), stop=(k == 5),
                        )
                    cols = slice(nch * NCH, (nch + 1) * NCH)
                    if m < 6:
                        # bias-add + f16 downcast drain on DVE (ACT is the
                        # busier engine); qkb broadcast along the free dim.
                        nc.vector.tensor_add(
                            out=qz_sb[0:64, m, 0, cols], in0=ps[0:64],
                            in1=qkb_sb[0:64, m : m + 1].to_broadcast([64, NCH]),
                        )
                        nc.vector.tensor_add(
                            out=qz_sb[64:128, m, 1, cols], in0=ps[64:128],
                            in1=qkb_sb[64:128, m : m + 1].to_broadcast([64, NCH]),
                        )
                    else:
                        nc.scalar.activation(
                            kT_sb[:, m - 6, cols], ps[:],
                            mybir.ActivationFunctionType.Copy)
                return run

            def mk_v_chunk(bt, ncb):
                def run():
                    b, t = divmod(bt, 2)
                    tbase, tsz = KT[t]
                    col0 = b * N_TOK + tbase
                    ps = psum.tile([128, 384], F32, tag="ps")
                    for k in range(6):
                        nc.tensor.matmul(
                            ps[:tsz],
                            lhsT=xT_sb[:, k, col0 : col0 + tsz],
                            rhs=wvT_sb[:, k, ncb * 384 : (ncb + 1) * 384],
                            start=(k == 0), stop=(k == 5),
                        )
                    src = ps[:tsz].rearrange("p (c j d) -> p c j d", c=3, j=2)
                    vbv = vb_sb[:tsz, ncb * 384 : (ncb + 1) * 384].rearrange(
                        "p (c j d) -> p c j d", c=3, j=2)
                    # both j-halves in one DVE op: the two 64-wide v slots
                    # sit at free offsets 0 and 129 of the vext row (stride
                    # 129 exactly covers [0:64] and [129:193]).
                    pear = vext_sb[:tsz, bt, 3 * ncb : 3 * ncb + 3, :]
                    dst = bass.AP(
                        tensor=pear.tensor, offset=pear.offset,
                        ap=list(pear.ap)[:-1] + [[129, 2], [1, 64]])
                    nc.vector.tensor_add(out=dst, in0=src, in1=vbv)
                return run

            def mk_scores(b, g, ets_all):
                def run():
                    q0 = b * N_TOK
                    for pig in (0, 1):
                        c = 2 * g + pig
                        ets = []
                        for t, (kbase, ksz) in enumerate(KT):
                            kcol = q0 + kbase
                            ps = psum.tile([128, 2 * N_TOK], F32, tag="ps")
                            nc.tensor.matmul(
                                ps[:ksz, :].rearrange("p (j q) -> p j q", j=2),
                                lhsT=kT_sb[:, c, kcol : kcol + ksz],
                                rhs=qz_sb[:, c, :, q0 : q0 + N_TOK],
                                start=True, stop=True,
                            )
                            et = exp_pool.tile([128, 2 * N_TOK], F16, tag="exp")
                            nc.scalar.activation(
                                et[:ksz], ps[:ksz], mybir.ActivationFunctionType.Exp)
                            rp = rpb0_sb if t == 0 else rpb1_sb
                            nc.vector.tensor_mul(et[:ksz], et[:ksz], rp[:ksz, c, :])
                            ets.append((et, ksz))
                        ets_all[(g, pig)] = ets
                    emit_proj_fill(1)
                return run

            def mk_pv(b, g, attn_sb, ets_all):
                def run():
                    gpar = (b * 3 + g) % 2
                    # PV psum for 2 pairs; 256-stride keeps each matmul
                    # region inside one PSUM bank.
                    pvg = psum_pv.tile([128, 2, 2, 256], F32, tag="pvg")
                    for pig in (0, 1):
                        c = 2 * g + pig
                        for j in (0, 1):
                            outap = pvg[:, pig, j, 0:N_TOK]
                            lo, hi = (0, 128) if j == 0 else (65, 193)
                            for t, (et, ksz) in enumerate(ets_all[(g, pig)]):
                                nc.tensor.matmul(
                                    outap,
                                    lhsT=vext_sb[:ksz, 2 * b + t, c, lo:hi],
                                    rhs=et[:ksz, j * N_TOK : (j + 1) * N_TOK],
                                    start=(t == 0), stop=(t == 1),
                                )
                    # softmax denominators: psum row 64 (even heads, j=0)
                    # and row 32 (odd heads, j=1).  ln(s) in f16 (same ACT
                    # table as Exp) lands both pigs' rows on partitions
                    # 0/32 of lns_sb; the deferred K=33 ones-matmul
                    # broadcasts 1/s partition-aligned with the PV output.
                    nc.scalar.activation(
                        lns_sb[0:1, gpar, :, :], pvg[64:65, :, 0, 0:N_TOK],
                        mybir.ActivationFunctionType.Ln)
                    nc.scalar.activation(
                        lns_sb[32:33, gpar, :, :], pvg[32:33, :, 1, 0:N_TOK],
                        mybir.ActivationFunctionType.Ln)
                    norm_fifo.append((b, g, gpar, attn_sb, pvg))
                    # the last two batches have no later work to hide norm
                    # latency behind -- emit eagerly so their proj chunks
                    # unlock as early as possible.
                    while len(norm_fifo) > (1 if b < 7 else 0):
                        emit_norm()
                    emit_proj_fill(1)
                return run

            def mk_attn_closures(b):
                attn_sb = attn_pool.tile([128, 6, N_TOK], F16, tag="attn")
                ets_all = {}
                cls = []
                for g in range(NPAIR // 2):
                    cls.append(mk_scores(b, g, ets_all))
                for g in range(NPAIR // 2):
                    cls.append(mk_pv(b, g, attn_sb, ets_all))
                return cls

            for nch in range(4):
                qc = [mk_qkv_chunk(nch, m) for m in range(12)]
                vc = [mk_v_chunk(bt, ncb)
                      for bt in range(4 * nch, 4 * nch + 4)
                      for ncb in range(2)]
                # v interleaves with the later qkv chunks (it only needs
                # xT + wvT, both early arrivals) so the PE has independent
                # work to absorb weight-DMA jitter.  The last block also
                # zippers its OWN batches' attention (6,7) in behind the
                # chunks they consume; each closure carries the minimum
                # number of gemm chunks that must be EMITTED first (the
                # dependency tracker only orders reads emitted after their
                # producers -- an early emission would be a silent race).
                if nch < 3:
                    gemm = qc[:6]
                    for i in range(8):
                        if i < 6:
                            gemm.append(qc[6 + i])
                        gemm.append(vc[i])
                else:
                    gemm = qc + [vc[0], vc[2], vc[1], vc[3],
                                 vc[4], vc[6], vc[5], vc[7]]
                attn = []
                if phases >= 3 and nch >= 1:
                    attn = [(0, cl) for cl in
                            mk_attn_closures(2 * (nch - 1))
                            + mk_attn_closures(2 * nch - 1)]
                    if nch == 3:
                        # closure order per batch: S0 S1 S2 P0 P1 P2
                        attn += list(zip(
                            [8, 10, 12, 14, 16, 16],
                            mk_attn_closures(6)))
                        attn += list(zip(
                            [8, 10, 12, 18, 20, 20],
                            mk_attn_closures(7)))
                # zipper: spread the attention closures through the gemm
                # chunks (gemm leads; both lists drain fully).
                na, ng = len(attn), len(gemm)
                ai = 0
                for gi, gcl in enumerate(gemm):
                    gcl()
                    while (ai < na and attn[ai][0] <= gi + 1
                           and ai * ng < na * (gi + 1)):
                        attn[ai][1]()
                        ai += 1
                while ai < na:
                    attn[ai][1]()
                    ai += 1
            while norm_fifo:
                emit_norm()
            if phases >= 4:
                emit_proj_fill(len(proj_fifo))
            if phases < 4:
                # debug: dump some qkT into out so the output is written
                dbg = out_pool.tile([128, TOK], F16, tag="dbg")
                nc.scalar.activation(dbg[:], kT_sb[:, 0, :],
                                     mybir.ActivationFunctionType.Copy)
                for r in range(0, DIM, 128):
                    nc.sync.dma_start(out.ap()[r : r + 128, :], dbg[:])
    return nc


_NC_CACHE = None


def _get_nc():
    global _NC_CACHE
    if _NC_CACHE is None:
        _NC_CACHE = build_nc()
    return _NC_CACHE


def _execute(inputs, trace=False):
    in_maps = _host_prepare(**inputs)
    nc = _get_nc()
    res = run_bass_kernel_spmd(nc, in_maps, core_ids=list(range(N_CORES)),
                               trace=trace)
    outs = [
        np.ascontiguousarray(res.results[c]["out"].T)
        .astype(np.float32)
        .reshape(B_LOC, N_TOK, DIM)
        for c in range(N_CORES)
    ]
    return np.concatenate(outs, axis=0), res


def kernel(**inputs) -> np.ndarray:
    out, _ = _execute(inputs, trace=False)
    return out

